# revision 36
# baseline (speedup 1.0000x reference)
# CATS-SwiGLU decode kernel for TRN2 (8 NeuronCores, SPMD tensor-parallel).
#
# Reference computation (decode path, B=S=1):
#   x1    = silu(x @ Wgatet)                  [1,1,dff]
#   flags = |x1| > threshold
#   z     = where(flags, (x @ Wup.T) * x1, 0) [1,1,dff]
#   out   = z @ Wdownt                        [1,1,d]
#
# Sharding: d_ff (11008) split across 8 cores (1376 rows each). Each core
# computes its z slice and a full-width partial down-projection; the host
# sums the 8 partials (the all-reduce of the TP hint, done on host).
#
# All three GEMVs run on the Vector engine as fused multiply+reduce
# (affine_mul_reduce) over weight tiles laid out rows-on-partitions:
#   gate: rows = d_ff slice of Wgatet.T   (host-transposed)
#   up:   rows = d_ff slice of Wup        (natural layout)
#   down: rows = d      of Wdownt.T      (host-transposed)
# DVE streams weights at ~490 GB/s, above per-core HBM (~360-430 GB/s), so
# the kernel is memory-bound at full fp32 precision.
#
# Schedule: gate/up chunks interleave so z for the first F-half is ready
# mid-stream; z is replicated across partitions on the idle TensorEngine
# (transpose-matmul to a PSUM row, copy to SBUF, broadcast-matmul into a
# PSUM [128, FSH] tile). The down-projection is split into the two F-half
# ranges: the first half's reduces overlap the tail of the weight stream,
# so only the short second-half reduces remain after the last DMA.
import sys

for _p in ("/opt/trn_rl_repo",):
    if _p not in sys.path:
        sys.path.insert(0, _p)

import numpy as np

import concourse.bass as bass
import concourse.tile as tile
from concourse import bacc, mybir
from concourse.bass_utils import run_bass_kernel_spmd
from concourse.masks import make_identity

D = 4096
FF = 11008
NCORES = 8
FSH = FF // NCORES          # 1376 rows of d_ff per core
NCH = (FSH + 127) // 128    # 11 chunks of <=128 rows
LAST = FSH - 128 * (NCH - 1)  # 96 rows in the last chunk
NDG = 16                    # down-proj groups: 2 d-chunks (256 rows) each
CSPLIT = 9                  # z batch 1 = chunks [0, 9); batch 2 stays tiny
FA = CSPLIT * 128           # 768
FB = FSH - FA               # 608
F32 = mybir.dt.float32

_CACHE = {}


def _bcast(ap, parts):
    """Replicate a 1-D AP across `parts` partitions (0-stride partition dim)."""
    return bass.AP(tensor=ap.tensor, offset=ap.offset, ap=[[0, parts]] + list(ap.ap))


def _build_nc():
    nc = bacc.Bacc("TRN2", target_bir_lowering=False, debug=False)

    x_d = nc.dram_tensor("x", [D], F32, kind="ExternalInput")
    wg_d = nc.dram_tensor("wg", [FSH, D], F32, kind="ExternalInput")
    wu_d = nc.dram_tensor("wu", [FSH, D], F32, kind="ExternalInput")
    wd_d = nc.dram_tensor("wd", [NDG, 128, 2 * FSH], F32, kind="ExternalInput")
    thr_d = nc.dram_tensor("thr", [1], F32, kind="ExternalInput")
    out_d = nc.dram_tensor("out", [128, 32], F32, kind="ExternalOutput")

    with tile.TileContext(nc) as tc:
        with (
            tc.tile_pool(name="const", bufs=1) as const_pool,
            tc.tile_pool(name="wpool", bufs=4) as wpool,
            tc.tile_pool(name="apool", bufs=5) as apool,
            tc.tile_pool(name="acts", bufs=1) as acts,
            tc.tile_pool(name="psum", bufs=1, space="PSUM") as psum,
        ):
            # Ring plan: the sync ring carries the 22 gate/up weight tiles
            # back-to-back from t=0 (the DVE-critical stream), then three
            # down tiles that fill the post-stream DMA bubble while the DVE
            # drains its backlog. Constants and the other down tiles go on
            # the scalar ring: the first four prefetch early, the rest
            # self-pace on slot frees.
            x_rep = const_pool.tile([128, D], F32)
            nc.scalar.dma_start(out=x_rep[:], in_=_bcast(x_d.ap(), 128))
            thr_sb = const_pool.tile([128, 1], F32)
            nc.scalar.dma_start(out=thr_sb[:], in_=_bcast(thr_d.ap(), 128))

            x1 = acts.tile([128, NCH], F32)  # gate pre-activation
            u = acts.tile([128, NCH], F32)   # up projection
            nc.vector.memset(x1[:], 0.0)
            nc.vector.memset(u[:], 0.0)

            # warm the sigmoid/abs ACT tables while the DMA stream runs
            warm = acts.tile([128, 1], F32)
            nc.scalar.activation(
                warm[:], thr_sb[:], mybir.ActivationFunctionType.Sigmoid
            )
            nc.scalar.activation(
                warm[:], thr_sb[:], mybir.ActivationFunctionType.Abs
            )

            # z replication machinery (TensorEngine)
            ident = const_pool.tile([128, 128], F32)
            make_identity(nc, ident[:])
            ones_row = const_pool.tile([1, 128], F32)
            nc.vector.memset(ones_row[:], 1.0)
            z_row_ps = psum.tile([1, NCH * 128], F32)
            z_row_sb = const_pool.tile([1, NCH * 128], F32)
            z_rep = psum.tile([128, NCH * 128], F32)
            batches = ((0, CSPLIT), (CSPLIT, NCH))

            def z_batch_compute(bi):
                c0, c1 = batches[bi]
                cs = slice(c0, c1)
                sg = acts.tile([128, NCH], F32, tag="sg", name="sg")
                nc.scalar.activation(
                    sg[:, cs], x1[:, cs], mybir.ActivationFunctionType.Sigmoid
                )
                x1s = acts.tile([128, NCH], F32, tag="x1s", name="x1s")
                nc.vector.tensor_mul(x1s[:, cs], x1[:, cs], sg[:, cs])
                absx = acts.tile([128, NCH], F32, tag="absx", name="absx")
                nc.scalar.activation(
                    absx[:, cs], x1s[:, cs], mybir.ActivationFunctionType.Abs
                )
                mask = acts.tile([128, NCH], F32, tag="mask", name="mask")
                nc.vector.tensor_scalar(
                    out=mask[:, cs],
                    in0=absx[:, cs],
                    scalar1=thr_sb[:],
                    scalar2=None,
                    op0=mybir.AluOpType.is_gt,
                )
                z = acts.tile([128, NCH], F32, tag="z", name="z")
                nc.vector.tensor_mul(z[:, cs], u[:, cs], x1s[:, cs])
                zm = acts.tile([128, NCH], F32, tag="zm", name="zm")
                nc.vector.tensor_mul(zm[:, cs], z[:, cs], mask[:, cs])
                return zm

            def z_batch_rep(bi, zm):
                c0, c1 = batches[bi]
                for c in range(c0, c1):
                    pc = 128 if c < NCH - 1 else LAST
                    fs = slice(c * 128, c * 128 + pc)
                    nc.tensor.matmul(
                        out=z_row_ps[0:1, fs],
                        lhsT=zm[:pc, c : c + 1],
                        rhs=ident[:pc, :pc],
                        start=True,
                        stop=True,
                    )
                    nc.scalar.copy(z_row_sb[0:1, fs], z_row_ps[0:1, fs])
                    nc.tensor.matmul(
                        out=z_rep[:, fs],
                        lhsT=ones_row[0:1, :],
                        rhs=z_row_sb[0:1, fs],
                        start=True,
                        stop=True,
                    )

            # gate and up GEMVs: acc[p, c] = sum_j W[c*128+p, j] * x[j].
            # All weight DMAs stay back-to-back on the sync ring; batch-1 z
            # compute + PE replication are emitted mid-up-loop so they
            # overlap the stream.
            zm1 = None
            for wi, (wdram, acc) in enumerate(((wg_d, x1), (wu_d, u))):
                for c in range(NCH):
                    p = 128 if c < NCH - 1 else LAST
                    wt = wpool.tile([128, D], F32, tag="w", name="wt")
                    nc.sync.dma_start(
                        out=wt[:p, :], in_=wdram.ap()[c * 128 : c * 128 + p, :]
                    )
                    nc.vector.affine_mul_reduce(
                        out=wt[:p, :],
                        accum_out=acc[:p, c : c + 1],
                        in0=wt[:p, :],
                        in1=x_rep[:p, :],
                        scale=1.0,
                        bias=0.0,
                    )
                    if wi == 1 and c == batches[0][1] - 1:
                        zm1 = z_batch_compute(0)
                        z_batch_rep(0, zm1)
            zm2 = z_batch_compute(1)
            z_batch_rep(1, zm2)

            # down projection: osb[p, c] = sum_f WdT[c*128+p, f] * z[f]
            osb = acts.tile([128, 32], F32)
            for g in range(NDG):
                dt_ = apool.tile([128, 2 * FSH], F32, tag="wd", name="dt_")
                ring = nc.sync if g in (2, 3, 4) else nc.scalar
                ring.dma_start(out=dt_[:], in_=wd_d.ap()[g])
                for h in range(2):
                    sl = slice(h * FSH, (h + 1) * FSH)
                    nc.vector.affine_mul_reduce(
                        out=dt_[:, sl],
                        accum_out=osb[:, 2 * g + h : 2 * g + h + 1],
                        in0=dt_[:, sl],
                        in1=z_rep[:, 0:FSH],
                        scale=1.0,
                        bias=0.0,
                    )
            nc.sync.dma_start(out=out_d.ap(), in_=osb[:])

    nc.compile()
    return nc


def _get_nc():
    if "nc" not in _CACHE:
        _CACHE["nc"] = _build_nc()
    return _CACHE["nc"]


def make_in_maps(x, Wup, Wgatet, Wdownt, threshold):
    """Shard full inputs into the 8 per-core input maps."""
    x_flat = np.ascontiguousarray(np.asarray(x, dtype=np.float32).reshape(D))
    thr = np.asarray(threshold, dtype=np.float32).reshape(1)
    Wup = np.asarray(Wup, dtype=np.float32)
    Wgatet = np.asarray(Wgatet, dtype=np.float32)
    Wdownt = np.asarray(Wdownt, dtype=np.float32)
    in_maps = []
    for i in range(NCORES):
        sl = slice(i * FSH, (i + 1) * FSH)
        wg = np.ascontiguousarray(Wgatet[:, sl].T)          # [FSH, D]
        wu = np.ascontiguousarray(Wup[sl, :])               # [FSH, D]
        wdt = np.ascontiguousarray(Wdownt[sl, :].T)         # [D, FSH]
        a = wdt.reshape(32, 128, FSH)
        wd = np.ascontiguousarray(
            np.concatenate([a[0::2], a[1::2]], axis=2)
        )                                                   # [NDG, 128, 2*FSH]
        in_maps.append({"x": x_flat, "wg": wg, "wu": wu, "wd": wd, "thr": thr})
    return in_maps


def run_sharded(x, Wup, Wgatet, Wdownt, threshold, trace=False, tmpdir=None):
    """Run on the 8 NeuronCores; returns (full_output, BassKernelResults)."""
    nc = _get_nc()
    in_maps = make_in_maps(x, Wup, Wgatet, Wdownt, threshold)
    res = run_bass_kernel_spmd(
        nc, in_maps, list(range(NCORES)), trace=trace, tmpdir=tmpdir
    )
    # un-shard: osb[p, c] holds partial_out[c*128 + p]; sum partials over cores
    acc = np.zeros(D, dtype=np.float64)
    for r in res.results:
        acc += r["out"].T.reshape(D).astype(np.float64)
    out = acc.astype(np.float32).reshape(1, 1, D)
    return out, res


def kernel(x, Wup, Wgatet, Wdownt, threshold):
    out, _ = run_sharded(x, Wup, Wgatet, Wdownt, threshold)
    return out


# revision 38
# speedup vs baseline: 1.0502x; 1.0502x over previous
# CATS-SwiGLU decode kernel for TRN2 (8 NeuronCores, SPMD tensor-parallel).
#
# Reference computation (decode path, B=S=1):
#   x1    = silu(x @ Wgatet)                  [1,1,dff]
#   flags = |x1| > threshold
#   z     = where(flags, (x @ Wup.T) * x1, 0) [1,1,dff]
#   out   = z @ Wdownt                        [1,1,d]
#
# Sharding: d_ff (11008) split across 8 cores (1376 rows each). Each core
# computes its z slice and a full-width partial down-projection; the host
# sums the 8 partials (the all-reduce of the TP hint, done on host).
#
# The gate/up GEMVs and most of the down GEMV run on the Vector engine as
# fused multiply+reduce (affine_mul_reduce) over weight tiles laid out
# rows-on-partitions (host-pretransposed where needed); DVE streams weights
# at ~444 GB/s, around per-core HBM rate, so the kernel is memory-bound at
# full fp32 precision.  z is replicated across partitions on the otherwise
# idle TensorEngine (transpose-matmul to a PSUM row, copy to SBUF,
# broadcast-matmul into PSUM).  The down-projection tail is split: d-chunks
# 0..19 reduce on DVE against the PSUM z_rep; d columns [2560, 4096) are
# computed on the TensorEngine (zm columns as stationary, natural-layout
# Wdownt as moving) so both engines drain the tail concurrently.
import sys

for _p in ("/opt/trn_rl_repo",):
    if _p not in sys.path:
        sys.path.insert(0, _p)

import numpy as np

import concourse.bass as bass
import concourse.tile as tile
from concourse import bacc, mybir
from concourse.bass_utils import run_bass_kernel_spmd
from concourse.masks import make_identity

D = 4096
FF = 11008
NCORES = 8
FSH = FF // NCORES          # 1376 rows of d_ff per core
NCH = (FSH + 127) // 128    # 11 chunks of <=128 rows
LAST = FSH - 128 * (NCH - 1)  # 96 rows in the last chunk
NDG = 10                    # DVE down-proj groups: 2 d-chunks (256 d) each
DPE0 = 2 * NDG * 128        # 2560: first d column of the PE share
DPE = D - DPE0              # 1536 PE-share columns (= 3 x 512)
CSPLIT = 9                  # z batch 1 = chunks [0, 9); batch 2 stays tiny
F32 = mybir.dt.float32

_CACHE = {}


def _bcast(ap, parts):
    """Replicate a 1-D AP across `parts` partitions (0-stride partition dim)."""
    return bass.AP(tensor=ap.tensor, offset=ap.offset, ap=[[0, parts]] + list(ap.ap))


def _build_nc():
    nc = bacc.Bacc("TRN2", target_bir_lowering=False, debug=False)

    x_d = nc.dram_tensor("x", [D], F32, kind="ExternalInput")
    wg_d = nc.dram_tensor("wg", [FSH, D], F32, kind="ExternalInput")
    wu_d = nc.dram_tensor("wu", [FSH, D], F32, kind="ExternalInput")
    wd_d = nc.dram_tensor("wd", [NDG, 128, 2 * FSH], F32, kind="ExternalInput")
    wp_d = nc.dram_tensor("wp", [FSH, DPE], F32, kind="ExternalInput")
    thr_d = nc.dram_tensor("thr", [1], F32, kind="ExternalInput")
    out_d = nc.dram_tensor("out", [128, 2 * NDG], F32, kind="ExternalOutput")
    outp_d = nc.dram_tensor("out_pe", [1, DPE], F32, kind="ExternalOutput")

    with tile.TileContext(nc) as tc:
        with (
            tc.tile_pool(name="const", bufs=1) as const_pool,
            tc.tile_pool(name="wpool", bufs=4) as wpool,
            tc.tile_pool(name="apool", bufs=7) as apool,
            tc.tile_pool(name="pepool", bufs=4) as pepool,
            tc.tile_pool(name="acts", bufs=1) as acts,
            tc.tile_pool(name="psum", bufs=1, space="PSUM") as psum,
        ):
            # constants on the scalar (qAct) ring so the weight stream on
            # the sync (qSP) ring starts at t=0
            x_rep = const_pool.tile([128, D], F32)
            nc.scalar.dma_start(out=x_rep[:], in_=_bcast(x_d.ap(), 128))
            thr_sb = const_pool.tile([128, 1], F32)
            nc.scalar.dma_start(out=thr_sb[:], in_=_bcast(thr_d.ap(), 128))

            x1 = acts.tile([128, NCH], F32)  # gate pre-activation
            u = acts.tile([128, NCH], F32)   # up projection
            zm = acts.tile([128, NCH], F32)  # masked z
            nc.vector.memset(x1[:], 0.0)
            nc.vector.memset(u[:], 0.0)

            # warm the sigmoid/abs ACT tables while the DMA stream runs
            warm = acts.tile([128, 1], F32)
            nc.scalar.activation(
                warm[:], thr_sb[:], mybir.ActivationFunctionType.Sigmoid
            )
            nc.scalar.activation(
                warm[:], thr_sb[:], mybir.ActivationFunctionType.Abs
            )

            # z replication machinery (TensorEngine)
            ident = const_pool.tile([128, 128], F32)
            make_identity(nc, ident[:])
            ones_row = const_pool.tile([1, 128], F32)
            nc.vector.memset(ones_row[:], 1.0)
            z_row_sb = const_pool.tile([1, NCH * 128], F32)
            z_rep = psum.tile([128, NCH * 128], F32)
            batches = ((0, CSPLIT), (CSPLIT, NCH))

            def z_batch_compute(bi):
                c0, c1 = batches[bi]
                cs = slice(c0, c1)
                sg = acts.tile([128, NCH], F32, tag="sg", name="sg")
                nc.scalar.activation(
                    sg[:, cs], x1[:, cs], mybir.ActivationFunctionType.Sigmoid
                )
                x1s = acts.tile([128, NCH], F32, tag="x1s", name="x1s")
                nc.vector.tensor_mul(x1s[:, cs], x1[:, cs], sg[:, cs])
                absx = acts.tile([128, NCH], F32, tag="absx", name="absx")
                nc.scalar.activation(
                    absx[:, cs], x1s[:, cs], mybir.ActivationFunctionType.Abs
                )
                mask = acts.tile([128, NCH], F32, tag="mask", name="mask")
                nc.vector.tensor_scalar(
                    out=mask[:, cs],
                    in0=absx[:, cs],
                    scalar1=thr_sb[:],
                    scalar2=None,
                    op0=mybir.AluOpType.is_gt,
                )
                z = acts.tile([128, NCH], F32, tag="z", name="z")
                nc.vector.tensor_mul(z[:, cs], u[:, cs], x1s[:, cs])
                nc.vector.tensor_mul(zm[:, cs], z[:, cs], mask[:, cs])

            def z_batch_rep(bi):
                c0, c1 = batches[bi]
                for c in range(c0, c1):
                    pc = 128 if c < NCH - 1 else LAST
                    fs = slice(c * 128, c * 128 + pc)
                    zrow = psum.tile(
                        [1, 128], F32, tag="zrow", name="zrow", bufs=2
                    )
                    nc.tensor.matmul(
                        out=zrow[0:1, 0:pc],
                        lhsT=zm[:pc, c : c + 1],
                        rhs=ident[:pc, :pc],
                        start=True,
                        stop=True,
                    )
                    nc.scalar.copy(z_row_sb[0:1, fs], zrow[0:1, 0:pc])
                    nc.tensor.matmul(
                        out=z_rep[:, fs],
                        lhsT=ones_row[0:1, :],
                        rhs=z_row_sb[0:1, fs],
                        start=True,
                        stop=True,
                    )

            # gate and up GEMVs: acc[p, c] = sum_j W[c*128+p, j] * x[j].
            # All weight DMAs stay back-to-back on the sync ring; batch-1 z
            # compute + PE replication are emitted mid-up-loop so they
            # overlap the stream.
            for wi, (wdram, acc) in enumerate(((wg_d, x1), (wu_d, u))):
                for c in range(NCH):
                    p = 128 if c < NCH - 1 else LAST
                    wt = wpool.tile([128, D], F32, tag="w", name="wt")
                    nc.sync.dma_start(
                        out=wt[:p, :], in_=wdram.ap()[c * 128 : c * 128 + p, :]
                    )
                    nc.vector.affine_mul_reduce(
                        out=wt[:p, :],
                        accum_out=acc[:p, c : c + 1],
                        in0=wt[:p, :],
                        in1=x_rep[:p, :],
                        scale=1.0,
                        bias=0.0,
                    )
                    if wi == 1 and c == CSPLIT - 1:
                        z_batch_compute(0)
                        z_batch_rep(0)
            z_batch_compute(1)
            z_batch_rep(1)

            # PE down share: out_pe[0, j] = sum_f zm_f * Wdownt[f, DPE0+j],
            # accumulated over the 11 F-chunks in PSUM.  Tiles stream on
            # the sync ring right after the gate/up weights.
            pe_out = psum.tile([1, DPE], F32, tag="peout")
            pe_tiles = []
            for fc in range(NCH):
                pc = 128 if fc < NCH - 1 else LAST
                pt = pepool.tile([128, DPE], F32, tag="pe", name=f"pe{fc}")
                pe_tiles.append(pt)
                nc.sync.dma_start(
                    out=pt[:pc, :], in_=wp_d.ap()[fc * 128 : fc * 128 + pc, :]
                )
            for fc in range(NCH):
                pc = 128 if fc < NCH - 1 else LAST
                for nb in range(DPE // 512):
                    ns = slice(nb * 512, (nb + 1) * 512)
                    nc.tensor.matmul(
                        out=pe_out[0:1, ns],
                        lhsT=zm[:pc, fc : fc + 1],
                        rhs=pe_tiles[fc][:pc, ns],
                        start=(fc == 0),
                        stop=(fc == NCH - 1),
                    )

            # DVE down share: osb[p, c] = sum_f WdT[c*128+p, f] * z[f]
            osb = acts.tile([128, 2 * NDG], F32)
            for g in range(NDG):
                dt_ = apool.tile([128, 2 * FSH], F32, tag="wd", name="dt_")
                ring = nc.sync if g in (4, 5, 6) else nc.scalar
                ring.dma_start(out=dt_[:], in_=wd_d.ap()[g])
                for h in range(2):
                    sl = slice(h * FSH, (h + 1) * FSH)
                    nc.vector.affine_mul_reduce(
                        out=dt_[:, sl],
                        accum_out=osb[:, 2 * g + h : 2 * g + h + 1],
                        in0=dt_[:, sl],
                        in1=z_rep[:, 0:FSH],
                        scale=1.0,
                        bias=0.0,
                    )
            nc.sync.dma_start(out=out_d.ap(), in_=osb[:])
            pe_out_sb = acts.tile([1, DPE], F32)
            nc.scalar.copy(pe_out_sb[:], pe_out[:])
            nc.scalar.dma_start(out=outp_d.ap(), in_=pe_out_sb[:])

    nc.compile()
    return nc


def _get_nc():
    if "nc" not in _CACHE:
        _CACHE["nc"] = _build_nc()
    return _CACHE["nc"]


def make_in_maps(x, Wup, Wgatet, Wdownt, threshold):
    """Shard full inputs into the 8 per-core input maps."""
    x_flat = np.ascontiguousarray(np.asarray(x, dtype=np.float32).reshape(D))
    thr = np.asarray(threshold, dtype=np.float32).reshape(1)
    Wup = np.asarray(Wup, dtype=np.float32)
    Wgatet = np.asarray(Wgatet, dtype=np.float32)
    Wdownt = np.asarray(Wdownt, dtype=np.float32)
    in_maps = []
    for i in range(NCORES):
        sl = slice(i * FSH, (i + 1) * FSH)
        wg = np.ascontiguousarray(Wgatet[:, sl].T)          # [FSH, D]
        wu = np.ascontiguousarray(Wup[sl, :])               # [FSH, D]
        wdt = np.ascontiguousarray(Wdownt[sl, 0:DPE0].T)    # [DPE0, FSH]
        a = wdt.reshape(2 * NDG, 128, FSH)
        wd = np.ascontiguousarray(
            np.concatenate([a[0::2], a[1::2]], axis=2)
        )                                                   # [NDG, 128, 2*FSH]
        wp = np.ascontiguousarray(Wdownt[sl, DPE0:])        # [FSH, DPE]
        in_maps.append(
            {"x": x_flat, "wg": wg, "wu": wu, "wd": wd, "wp": wp, "thr": thr}
        )
    return in_maps


def run_sharded(x, Wup, Wgatet, Wdownt, threshold, trace=False, tmpdir=None):
    """Run on the 8 NeuronCores; returns (full_output, BassKernelResults)."""
    nc = _get_nc()
    in_maps = make_in_maps(x, Wup, Wgatet, Wdownt, threshold)
    res = run_bass_kernel_spmd(
        nc, in_maps, list(range(NCORES)), trace=trace, tmpdir=tmpdir
    )
    # un-shard: osb[p, c] holds partial_out[c*128 + p] for d < DPE0 and
    # out_pe[0, j] holds partial_out[DPE0 + j]; sum partials over cores.
    acc = np.zeros(D, dtype=np.float64)
    for r in res.results:
        acc[0:DPE0] += r["out"].T.reshape(DPE0).astype(np.float64)
        acc[DPE0:] += r["out_pe"][0].astype(np.float64)
    out = acc.astype(np.float32).reshape(1, 1, D)
    return out, res


def kernel(x, Wup, Wgatet, Wdownt, threshold):
    out, _ = run_sharded(x, Wup, Wgatet, Wdownt, threshold)
    return out
